# revision 32
# baseline (speedup 1.0000x reference)
"""Trainium2 Bass kernel: class-routed 2-layer MLP (MoE-style routing).

    out[b] = W2[y[b]] . tanh(W1[y[b]] @ Z[b] + b1[y[b]]) + b2[y[b]]

Sharding strategy (expert/class sharding, not batch sharding):
  - Classes present in y are assigned to the 8 cores by greedy
    load-balancing; samples are routed on the host to the core owning
    their class.
  - On each core the kernel iterates over "slots": one slot = one unique
    class plus up to S of its routed samples. Host packs, per slot,
    the class's transposed W1 row ([F,H] layout, f on partitions, fp16)
    so the device program is a fully static stream: one contiguous DMA
    per slot group carrying W1cT plus that slot's Z columns -> 16 matmuls
    -> tanh(+b1) -> small matmul with W2 -> output slot.
  - Deduplication: each class's W1 row is read from HBM once globally
    (vs once per sample for the naive gather), which is what the
    memory-bound roofline wants. Measured ~187 us on 8 cores (DMA
    bursts >400 GB/s/core, ~340 avg incl. head/tail), rel err ~1e-3.

All routing/gather/scatter is host-side numpy baked into the input
layout; the device NEFF is identical across cores (SPMD) and contains no
data-dependent control flow.
"""

import os
import numpy as np

N_CORES = 8
S = 8                       # sample capacity per class-slot
F = 512                     # feature dim (layer-1 contraction)
H = 512                     # hidden dim
FT = F // 128               # f-tiles
HT = H // 128               # h-tiles

# "float32" / "float16" / "bfloat16" for the streamed W1/W2/Z tensors.
# float16 keeps ~1e-3 relative accuracy while halving HBM traffic and
# running single-pass matmuls (fp32 matmuls are two-pass HI/LO on trn2).
W_DTYPE = os.environ.get("KERNEL_W_DTYPE", "float16")
DMA_GROUP = 4   # weight slots per dma_start (2 MB @ fp16)
OUT_GROUP = 16  # slots sharing one PSUM output bank before copy-out
L2_LAG = 1      # groups between h-matmuls and the slot's layer-2 matmul

# Set by kernel() after each run (ns, from neuron-profile; None w/o trace).
LAST_EXEC_TIME_NS = None
LAST_MEAN_EXEC_TIME_NS = None

_PROGRAM_CACHE = {}


def _route(y):
    """Group samples by class, balance classes across cores, build slots.

    Each present class contributes ceil(n_c / S) slots (one slot = one
    class + up to S samples). Classes are assigned to cores greedily
    (most slots first -> least-loaded core) so the per-core slot count —
    which sets the per-core HBM traffic — is near-uniform.

    Returns (slots_per_core, K): slots_per_core[m] is a list of
    (class_id, sample_index_array); K = max slot count over cores.
    """
    order = np.argsort(y, kind="stable")
    ys = y[order]
    uniq, starts, counts = np.unique(ys, return_index=True, return_counts=True)
    class_slots = []  # (n_slots, class_id, sample_idx_array)
    for u, s0, n in zip(uniq, starts, counts):
        class_slots.append((-(-int(n) // S), int(u), order[s0 : s0 + n]))
    class_slots.sort(key=lambda t: -t[0])
    loads = [0] * N_CORES
    slots_per_core = [[] for _ in range(N_CORES)]
    for nslots, cls, sidx in class_slots:
        m = loads.index(min(loads))
        loads[m] += nslots
        for j in range(0, len(sidx), S):
            slots_per_core[m].append((cls, sidx[j : j + S]))
    K = max(1, max(len(s) for s in slots_per_core))
    return slots_per_core, K


def _build_program(K, bias_zero):
    import concourse.mybir as mybir
    import concourse.tile as tile
    from concourse import bacc

    f32 = mybir.dt.float32
    wdt = {
        "float32": f32,
        "float16": mybir.dt.float16,
        "bfloat16": mybir.dt.bfloat16,
    }[W_DTYPE]
    G = DMA_GROUP
    assert K % G == 0

    C = FT * H  # per-slot stream columns: W1cT only (Z is resident)
    # Variable group schedule: two small leading groups so the first
    # weights complete (and PE starts) ~15us earlier; 4-slot groups after.
    sizes = [2, 2] + [4] * ((K - 4) // 4) if K >= 8 else [K]
    assert sum(sizes) == K
    NG = len(sizes)
    offs = [0]
    for s_ in sizes:
        offs.append(offs[-1] + s_)
    HS = HT * S

    nc = bacc.Bacc("TRN2", debug=False)
    wz = nc.dram_tensor("wz", [128, K * C], wdt, kind="ExternalInput")
    zs = nc.dram_tensor("zs", [128, FT * K * S], wdt, kind="ExternalInput")
    b1s = None
    if not bias_zero:
        b1s = nc.dram_tensor("b1s", [128, K * HT], f32, kind="ExternalInput")
    w2s = nc.dram_tensor("w2s", [128, K * HT], wdt, kind="ExternalInput")
    # Layer-2 leaves the device as [HT, K*HT*S] blocks: one matmul per slot
    # with lhsT = the slot's W2 column block [128, HT] produces [HT, HT*S];
    # the host sums the block diagonal out[k,s] = sum_ht o[ht, k, ht*S+s].
    out = nc.dram_tensor("out", [HT, K * HS], f32, kind="ExternalOutput")

    with tile.TileContext(nc) as tc:
        with (
            tc.tile_pool(name="consts", bufs=1) as cpool,
            tc.tile_pool(name="wp", bufs=9) as wpool,
            tc.tile_pool(name="thp", bufs=4) as thpool,
            tc.tile_pool(name="scp", bufs=3) as scpool,
            tc.tile_pool(name="hps", bufs=5, space="PSUM") as hpool,
            tc.tile_pool(name="ops", bufs=3, space="PSUM") as opool,
        ):
            z_sb = cpool.tile([128, FT, K, S], wdt)
            b1_sb = None
            if not bias_zero:
                b1_sb = cpool.tile([128, K * HT], f32)
            w2_sb = cpool.tile([128, K * HT], wdt)
            out_sb = cpool.tile([HT, K * HS], f32)

            # Weight-group DMAs alternate the SP / ACT HWDGE rings. The
            # ACT engine also runs the tanhs, and a sequencer processes its
            # stream in order — so each ring's dma_start is EMITTED `HOIST`
            # groups before its consumers. That keeps ring issue ahead of
            # compute by ~HOIST groups: a compute hiccup no longer starves
            # descriptor issue (the baseline's mid-run DMA sag).
            def issue_group(j):
                # Weight groups alternate the SP/ACT HWDGE rings. With the
                # rational tanh on DVE (below), NEITHER ring's sequencer
                # carries any compute-dependent instruction, so a dma_start
                # blocking (ring-full or frame-free) can never stall
                # compute.
                eng = nc.sync if j % 2 == 0 else nc.scalar
                w_sb = wpool.tile([128, sizes[j] * C], wdt, name="w_sb")
                eng.dma_start(w_sb[:], wz[:, offs[j] * C : offs[j + 1] * C])
                return w_sb

            # Pre-issue EVERY weight group. The HWDGE ring holds 4
            # in-flight dma_starts; the 5th blocks the sequencer until the
            # oldest transfer completes, so each ring self-paces at full
            # rate. Nothing else lives on these sequencers, so the blocking
            # never stalls compute.
            w_tiles = {0: issue_group(0), 1: issue_group(1)}
            # Residents after the two small lead groups: Z rides the SP
            # ring (its completion gates the very first matmul), W2/b1
            # behind it. Then the rest of the stream.
            nc.sync.dma_start(z_sb[:], zs[:])
            if not bias_zero:
                nc.sync.dma_start(b1_sb[:], b1s[:])
            nc.sync.dma_start(w2_sb[:], w2s[:])
            for j in range(2, NG):
                w_tiles[j] = issue_group(j)

            th_tiles = {}
            state = {"o_ps": None}
            flush_q = []

            def emit_flush(k0, n, o_ps):
                # PSUM->SBUF copy on DVE + SWDGE writeback, deferred ~2
                # groups past the block's last layer-2 matmul so the copy's
                # wait never stalls the tanh chains behind it in DVE's
                # in-order stream.
                nc.vector.tensor_copy(out_sb[:, k0 * HS : k0 * HS + n], o_ps[:, :n])
                nc.gpsimd.dma_start(
                    out[:, k0 * HS : k0 * HS + n],
                    out_sb[:, k0 * HS : k0 * HS + n],
                )

            def emit_l2(gi):
                # Layer-2 matmuls for group gi, emitted L2_LAG groups after
                # its h-matmuls. PE's queue is in-order, so emitting these
                # right after group gi's h-matmuls would stall PE on the
                # PE->ACT->PE tanh round-trip every group (~1.5us/group of
                # dead time, the mid-run DMA sag). By the lagged emission
                # point tanh(gi) finished long ago and PE never blocks.
                th_g = th_tiles.pop(gi)
                while flush_q and flush_q[0][0] <= gi - 2:
                    emit_flush(*flush_q.pop(0)[1])
                for g2 in range(sizes[gi]):
                    k2 = offs[gi] + g2
                    if k2 >= K:
                        break
                    if k2 % OUT_GROUP == 0:
                        state["o_ps"] = opool.tile(
                            [HT, OUT_GROUP * HS], f32, name="o_ps"
                        )
                    o_ps = state["o_ps"]
                    jo = (k2 % OUT_GROUP) * HS
                    nc.tensor.matmul(
                        o_ps[:, jo : jo + HS],
                        w2_sb[:, k2 * HT : (k2 + 1) * HT],
                        th_g[:, g2 * HS : (g2 + 1) * HS],
                        start=True,
                        stop=True,
                    )
                    if k2 % OUT_GROUP == OUT_GROUP - 1 or k2 == K - 1:
                        k0 = (k2 // OUT_GROUP) * OUT_GROUP
                        n = (k2 - k0 + 1) * HS
                        flush_q.append((gi, (k0, n, o_ps)))

            h_ps = None
            th_sb = None
            for k in range(K):
                j = 0
                while offs[j + 1] <= k:
                    j += 1
                g = k - offs[j]
                gsz = sizes[j]
                if g == 0:
                    w_sb = w_tiles.pop(j)
                    h_ps = hpool.tile([128, gsz * HS], f32, name="h_ps")
                    th_sb = thpool.tile([128, gsz * HS], wdt, name="th_sb")
                    th_tiles[j] = th_sb
                o = g * C

                for ht in range(HT):
                    for ft in range(FT):
                        nc.tensor.matmul(
                            h_ps[:, (g * HT + ht) * S : (g * HT + ht + 1) * S],
                            w_sb[:, o + ft * H + ht * 128 : o + ft * H + (ht + 1) * 128],
                            z_sb[:, ft, k, :],
                            start=(ft == 0),
                            stop=(ft == FT - 1),
                        )

                if g == gsz - 1:
                    if bias_zero:
                        # tanh on the vector engine as a clamped Pade(3,2)
                        # rational: t = clamp(x, +-3); tanh ~ t*(27+t^2) /
                        # (27+9t^2). Exact 1.0 at the clamp boundary; max
                        # error ~5e-3 mid-range, ~1e-3 rms contribution to
                        # the output - same order as the fp16 weight noise.
                        # Keeps the ACT engine free so its HWDGE ring is
                        # pure DMA.
                        sc = scpool.tile([128, 5, gsz * HS], f32, name="sc")
                        t, u, dn, r, nm = (sc[:, i] for i in range(5))
                        nc.vector.tensor_scalar(
                            t, h_ps[:], 3.0, -3.0,
                            mybir.AluOpType.min, mybir.AluOpType.max,
                        )
                        nc.vector.tensor_mul(u, t, t)
                        nc.vector.tensor_scalar(
                            dn, u, 9.0, 27.0,
                            mybir.AluOpType.mult, mybir.AluOpType.add,
                        )
                        nc.vector.reciprocal_approx_fast(r, dn)
                        nc.vector.scalar_tensor_tensor(
                            nm, u, 27.0, t,
                            mybir.AluOpType.add, mybir.AluOpType.mult,
                        )
                        nc.vector.tensor_mul(th_sb[:], nm, r)
                    else:
                        for g2 in range(gsz):
                            for ht in range(HT):
                                k2 = (k - (gsz - 1)) + g2
                                nc.scalar.activation(
                                    th_sb[:, (g2 * HT + ht) * S : (g2 * HT + ht + 1) * S],
                                    h_ps[:, (g2 * HT + ht) * S : (g2 * HT + ht + 1) * S],
                                    mybir.ActivationFunctionType.Tanh,
                                    bias=b1_sb[:, k2 * HT + ht : k2 * HT + ht + 1],
                                )
                    if j >= L2_LAG:
                        emit_l2(j - L2_LAG)

            for gi in range(max(0, NG - L2_LAG), NG):
                emit_l2(gi)
            while flush_q:
                emit_flush(*flush_q.pop(0)[1])

    nc.compile()
    return nc


def _install_profile_hook():
    """Register the axon NTFF profiling hook if the image lacks
    antenv.axon_hooks (degrades to no trace if anything is missing)."""
    import sys
    import types

    try:
        from antenv.axon_hooks import get_axon_ntff_profile_hook  # noqa: F401

        return
    except ImportError:
        pass
    try:
        import antenv
        from trn_agent_boot.trn_boot import _ntff_profile_via_ctypes

        so = "/opt/axon/libaxon_pjrt.so"
        if not os.path.exists(so):
            return
        mod = types.ModuleType("antenv.axon_hooks")
        holder = [None]
        mod.set_axon_ntff_profile_hook = lambda h: holder.__setitem__(0, h)
        mod.get_axon_ntff_profile_hook = lambda: holder[0]
        sys.modules["antenv.axon_hooks"] = mod
        antenv.axon_hooks = mod
        mod.set_axon_ntff_profile_hook(_ntff_profile_via_ctypes(so))
    except Exception:
        pass


def kernel(Z, y, W1, b1, W2, b2):
    global LAST_EXEC_TIME_NS, LAST_MEAN_EXEC_TIME_NS
    import sys

    if "jax" not in sys.modules:
        os.environ.setdefault("JAX_PLATFORMS", "axon")
    from concourse.bass_utils import run_bass_kernel_spmd

    Z = np.asarray(Z, dtype=np.float32)
    y = np.asarray(y).astype(np.int64)
    W1 = np.asarray(W1, dtype=np.float32)
    b1 = np.asarray(b1, dtype=np.float32)
    W2 = np.asarray(W2, dtype=np.float32)
    b2 = np.asarray(b2, dtype=np.float32)
    B = Z.shape[0]
    assert Z.shape == (B, F) and W1.shape[1:] == (H, F)

    if W_DTYPE == "float32":
        wnp = np.float32
    elif W_DTYPE == "float16":
        wnp = np.float16
    else:
        import ml_dtypes

        wnp = ml_dtypes.bfloat16

    slots_per_core, K = _route(y)
    K = ((K + DMA_GROUP - 1) // DMA_GROUP) * DMA_GROUP
    bias_zero = not np.any(b1)
    key = (K, W_DTYPE, bias_zero)
    if key not in _PROGRAM_CACHE:
        _PROGRAM_CACHE[key] = _build_program(K, bias_zero)
    nc = _PROGRAM_CACHE[key]

    Zt = np.ascontiguousarray(Z.T)  # [F, B]
    C = FT * H
    in_maps = []
    for m in range(N_CORES):
        slots = slots_per_core[m]
        cls_list = np.array(
            [c for c, _ in slots] + [0] * (K - len(slots)), dtype=np.int64
        )
        # Flat weight stream: wz[p, k*C + ft*H + h] = W1[cls_k, h, ft*128+p]
        wzm = np.empty((128, K * C), wnp)
        np.copyto(
            wzm.reshape(128, K, FT, H),
            W1[cls_list].reshape(K, H, FT, 128).transpose(3, 0, 2, 1),
            casting="same_kind",
        )
        # Resident routed Z: zs[p, ft, k, s] = Z[sample_s_of_slot_k, ft*128+p]
        zsm = np.zeros((128, FT, K, S), wnp)
        for k, (_, sidx) in enumerate(slots):
            zsm[:, :, k, : len(sidx)] = (
                Zt[:, sidx].reshape(FT, 128, len(sidx)).transpose(1, 0, 2)
            )
        # b1s[p, k*HT + ht] = b1[cls_k, ht*128 + p]; same layout for w2s
        w2sm = (
            W2[cls_list]
            .reshape(K, HT, 128)
            .transpose(2, 0, 1)
            .astype(wnp)
            .reshape(128, K * HT)
        )
        im = {"wz": wzm, "zs": zsm.reshape(128, FT * K * S), "w2s": w2sm}
        if not bias_zero:
            im["b1s"] = np.ascontiguousarray(
                b1[cls_list].reshape(K, HT, 128).transpose(2, 0, 1)
            ).reshape(128, K * HT)
        in_maps.append(im)

    trace = os.environ.get("KERNEL_TRACE", "0") == "1"
    if trace:
        _install_profile_hook()
    res = run_bass_kernel_spmd(
        nc, in_maps, core_ids=list(range(N_CORES)), trace=trace
    )
    LAST_EXEC_TIME_NS = res.exec_time_ns
    LAST_MEAN_EXEC_TIME_NS = res.mean_exec_time_ns

    out = np.empty(B, dtype=np.float32)
    for m in range(N_CORES):
        # o[ht, k, ht2, s]: slot k's layer-2 matmul block; the block
        # diagonal ht2 == ht holds W2[cls,ht-block] . th[ht-block, s].
        o = np.asarray(res.results[m]["out"]).reshape(HT, K, HT, S)
        osum = np.einsum("hkhs->ks", o)
        for k, (_, sidx) in enumerate(slots_per_core[m]):
            out[sidx] = osum[k, : len(sidx)]
    out += b2[y]
    return out



# revision 33
# speedup vs baseline: 1.0663x; 1.0663x over previous
"""Trainium2 Bass kernel: class-routed 2-layer MLP (MoE-style routing).

    out[b] = W2[y[b]] . tanh(W1[y[b]] @ Z[b] + b1[y[b]]) + b2[y[b]]

Sharding strategy (expert/class sharding, not batch sharding):
  - Classes present in y are assigned to the 8 cores by greedy
    load-balancing; samples are routed on the host to the core owning
    their class.
  - On each core the kernel iterates over "slots": one slot = one unique
    class plus up to S of its routed samples. The class's transposed W1
    row ([F,H] layout, f on partitions, fp16) is streamed from HBM in
    2-slot groups alternating the SP/ACT HWDGE rings; the routed Z
    samples and the W2 rows are small one-time residents.
  - Deduplication: each class's W1 row is read from HBM once globally
    (vs once per sample for the naive gather), which is what the
    memory-bound roofline wants.
  - Layer 2 is ONE matmul per slot: lhsT = the slot's W2 column block
    [128, HT] against th [128, HT*S] gives [HT, HT*S] in PSUM; the host
    sums the block diagonal. These matmuls are emitted L2_LAG groups
    late so PE's in-order queue never stalls on the PE->ACT tanh
    round-trip (that stall was the dominant serialization: ~3us/group).
  - Weight dma_starts are emitted HOIST groups ahead of their consumers
    and out-writebacks ride gpsimd/DVE, keeping descriptor issue ahead
    of compute. Measured 180.4us on 8 cores, rel err ~1e-3
    (fp16-quantization dominated; HBM roofline for the 57 MB/core
    stream is ~145us).

All routing/gather/scatter is host-side numpy baked into the input
layout; the device NEFF is identical across cores (SPMD) and contains no
data-dependent control flow.
"""

import os
import numpy as np

N_CORES = 8
S = 8                       # sample capacity per class-slot
F = 512                     # feature dim (layer-1 contraction)
H = 512                     # hidden dim
FT = F // 128               # f-tiles
HT = H // 128               # h-tiles

# "float32" / "float16" / "bfloat16" for the streamed W1/W2/Z tensors.
# float16 keeps ~1e-3 relative accuracy while halving HBM traffic and
# running single-pass matmuls (fp32 matmuls are two-pass HI/LO on trn2).
W_DTYPE = os.environ.get("KERNEL_W_DTYPE", "float16")
DMA_GROUP = 2   # weight slots per dma_start (1 MB @ fp16)
OUT_GROUP = 16  # slots sharing one PSUM output bank before copy-out
HOIST = 8       # groups of DMA-issue lead over the compute emission point
L2_LAG = 2      # groups between h-matmuls and the slot's layer-2 matmul

# Set by kernel() after each run (ns, from neuron-profile; None w/o trace).
LAST_EXEC_TIME_NS = None
LAST_MEAN_EXEC_TIME_NS = None

_PROGRAM_CACHE = {}


def _route(y):
    """Group samples by class, balance classes across cores, build slots.

    Each present class contributes ceil(n_c / S) slots (one slot = one
    class + up to S samples). Classes are assigned to cores greedily
    (most slots first -> least-loaded core) so the per-core slot count —
    which sets the per-core HBM traffic — is near-uniform.

    Returns (slots_per_core, K): slots_per_core[m] is a list of
    (class_id, sample_index_array); K = max slot count over cores.
    """
    order = np.argsort(y, kind="stable")
    ys = y[order]
    uniq, starts, counts = np.unique(ys, return_index=True, return_counts=True)
    class_slots = []  # (n_slots, class_id, sample_idx_array)
    for u, s0, n in zip(uniq, starts, counts):
        class_slots.append((-(-int(n) // S), int(u), order[s0 : s0 + n]))
    class_slots.sort(key=lambda t: -t[0])
    loads = [0] * N_CORES
    slots_per_core = [[] for _ in range(N_CORES)]
    for nslots, cls, sidx in class_slots:
        m = loads.index(min(loads))
        loads[m] += nslots
        for j in range(0, len(sidx), S):
            slots_per_core[m].append((cls, sidx[j : j + S]))
    K = max(1, max(len(s) for s in slots_per_core))
    return slots_per_core, K


def _build_program(K, bias_zero):
    import concourse.mybir as mybir
    import concourse.tile as tile
    from concourse import bacc

    f32 = mybir.dt.float32
    wdt = {
        "float32": f32,
        "float16": mybir.dt.float16,
        "bfloat16": mybir.dt.bfloat16,
    }[W_DTYPE]
    G = DMA_GROUP
    assert K % G == 0

    C = FT * H  # per-slot stream columns: W1cT only (Z is resident)
    NG = K // G
    HS = HT * S

    nc = bacc.Bacc("TRN2", debug=False)
    wz = nc.dram_tensor("wz", [NG, 128, G * C], wdt, kind="ExternalInput")
    zs = nc.dram_tensor("zs", [128, FT * K * S], wdt, kind="ExternalInput")
    b1s = None
    if not bias_zero:
        b1s = nc.dram_tensor("b1s", [128, K * HT], f32, kind="ExternalInput")
    w2s = nc.dram_tensor("w2s", [128, K * HT], wdt, kind="ExternalInput")
    # Layer-2 leaves the device as [HT, K*HT*S] blocks: one matmul per slot
    # with lhsT = the slot's W2 column block [128, HT] produces [HT, HT*S];
    # the host sums the block diagonal out[k,s] = sum_ht o[ht, k, ht*S+s].
    out = nc.dram_tensor("out", [HT, K * HS], f32, kind="ExternalOutput")

    with tile.TileContext(nc) as tc:
        with (
            tc.tile_pool(name="consts", bufs=1) as cpool,
            tc.tile_pool(name="wp", bufs=18) as wpool,
            tc.tile_pool(name="thp", bufs=4) as thpool,
            tc.tile_pool(name="hps", bufs=4, space="PSUM") as hpool,
            tc.tile_pool(name="ops", bufs=2, space="PSUM") as opool,
        ):
            # Residents: routed Z, biases, W2. Loaded once on the gpsimd
            # (SWDGE) ring, overlapping the framework preamble.
            z_sb = cpool.tile([128, FT, K, S], wdt)
            nc.gpsimd.dma_start(z_sb[:], zs[:])
            if not bias_zero:
                b1_sb = cpool.tile([128, K * HT], f32)
                nc.gpsimd.dma_start(b1_sb[:], b1s[:])
            w2_sb = cpool.tile([128, K * HT], wdt)
            nc.gpsimd.dma_start(w2_sb[:], w2s[:])
            out_sb = cpool.tile([HT, K * HS], f32)

            # Weight-group DMAs alternate the SP / ACT HWDGE rings. The
            # ACT engine also runs the tanhs, and a sequencer processes its
            # stream in order — so each ring's dma_start is EMITTED `HOIST`
            # groups before its consumers. That keeps ring issue ahead of
            # compute: a compute hiccup no longer starves descriptor issue
            # (the baseline's mid-run DMA sag).
            def issue_group(j):
                eng = nc.sync if j % 2 == 0 else nc.scalar
                w_sb = wpool.tile([128, G * C], wdt, name="w_sb")
                eng.dma_start(w_sb[:], wz[j])
                return w_sb

            w_tiles = {}
            for j in range(min(HOIST, NG)):
                w_tiles[j] = issue_group(j)

            th_tiles = {}
            state = {"o_ps": None}

            def emit_l2(gi):
                # Layer-2 matmuls for group gi, emitted L2_LAG groups after
                # its h-matmuls. PE's queue is in-order, so emitting these
                # right after group gi's h-matmuls would stall PE on the
                # PE->ACT->PE tanh round-trip every group (~1.5us/group of
                # dead time). By the lagged emission point tanh(gi)
                # finished long ago and PE never blocks.
                th_g = th_tiles.pop(gi)
                for g2 in range(G):
                    k2 = gi * G + g2
                    if k2 >= K:
                        break
                    if k2 % OUT_GROUP == 0:
                        state["o_ps"] = opool.tile(
                            [HT, OUT_GROUP * HS], f32, name="o_ps"
                        )
                    o_ps = state["o_ps"]
                    jo = (k2 % OUT_GROUP) * HS
                    nc.tensor.matmul(
                        o_ps[:, jo : jo + HS],
                        w2_sb[:, k2 * HT : (k2 + 1) * HT],
                        th_g[:, g2 * HS : (g2 + 1) * HS],
                        start=True,
                        stop=True,
                    )
                    if k2 % OUT_GROUP == OUT_GROUP - 1 or k2 == K - 1:
                        # PSUM->SBUF copy on the otherwise-idle DVE;
                        # writeback via gpsimd SWDGE. Neither weight
                        # ring ever waits on compute completion.
                        k0 = (k2 // OUT_GROUP) * OUT_GROUP
                        n = (k2 - k0 + 1) * HS
                        nc.vector.tensor_copy(
                            out_sb[:, k0 * HS : k0 * HS + n], o_ps[:, :n]
                        )
                        nc.gpsimd.dma_start(
                            out[:, k0 * HS : k0 * HS + n],
                            out_sb[:, k0 * HS : k0 * HS + n],
                        )

            h_ps = None
            th_sb = None
            for k in range(K):
                j, g = divmod(k, G)
                if g == 0:
                    if j + HOIST < NG:
                        w_tiles[j + HOIST] = issue_group(j + HOIST)
                    w_sb = w_tiles.pop(j)
                    h_ps = hpool.tile([128, G * HS], f32, name="h_ps")
                    th_sb = thpool.tile([128, G * HS], wdt, name="th_sb")
                    th_tiles[j] = th_sb
                o = g * C

                for ht in range(HT):
                    for ft in range(FT):
                        nc.tensor.matmul(
                            h_ps[:, (g * HT + ht) * S : (g * HT + ht + 1) * S],
                            w_sb[:, o + ft * H + ht * 128 : o + ft * H + (ht + 1) * 128],
                            z_sb[:, ft, k, :],
                            start=(ft == 0),
                            stop=(ft == FT - 1),
                        )

                if g == G - 1:
                    if bias_zero:
                        nc.scalar.activation(
                            th_sb[:], h_ps[:], mybir.ActivationFunctionType.Tanh
                        )
                    else:
                        for g2 in range(G):
                            for ht in range(HT):
                                k2 = (k - (G - 1)) + g2
                                nc.scalar.activation(
                                    th_sb[:, (g2 * HT + ht) * S : (g2 * HT + ht + 1) * S],
                                    h_ps[:, (g2 * HT + ht) * S : (g2 * HT + ht + 1) * S],
                                    mybir.ActivationFunctionType.Tanh,
                                    bias=b1_sb[:, k2 * HT + ht : k2 * HT + ht + 1],
                                )
                    if j >= L2_LAG:
                        emit_l2(j - L2_LAG)

            for gi in range(max(0, NG - L2_LAG), NG):
                emit_l2(gi)

    nc.compile()
    return nc


def _install_profile_hook():
    """Register the axon NTFF profiling hook if the image lacks
    antenv.axon_hooks (degrades to no trace if anything is missing)."""
    import sys
    import types

    try:
        from antenv.axon_hooks import get_axon_ntff_profile_hook  # noqa: F401

        return
    except ImportError:
        pass
    try:
        import antenv
        from trn_agent_boot.trn_boot import _ntff_profile_via_ctypes

        so = "/opt/axon/libaxon_pjrt.so"
        if not os.path.exists(so):
            return
        mod = types.ModuleType("antenv.axon_hooks")
        holder = [None]
        mod.set_axon_ntff_profile_hook = lambda h: holder.__setitem__(0, h)
        mod.get_axon_ntff_profile_hook = lambda: holder[0]
        sys.modules["antenv.axon_hooks"] = mod
        antenv.axon_hooks = mod
        mod.set_axon_ntff_profile_hook(_ntff_profile_via_ctypes(so))
    except Exception:
        pass


def kernel(Z, y, W1, b1, W2, b2):
    global LAST_EXEC_TIME_NS, LAST_MEAN_EXEC_TIME_NS
    import sys

    if "jax" not in sys.modules:
        os.environ.setdefault("JAX_PLATFORMS", "axon")
    from concourse.bass_utils import run_bass_kernel_spmd

    Z = np.asarray(Z, dtype=np.float32)
    y = np.asarray(y).astype(np.int64)
    W1 = np.asarray(W1, dtype=np.float32)
    b1 = np.asarray(b1, dtype=np.float32)
    W2 = np.asarray(W2, dtype=np.float32)
    b2 = np.asarray(b2, dtype=np.float32)
    B = Z.shape[0]
    assert Z.shape == (B, F) and W1.shape[1:] == (H, F)

    if W_DTYPE == "float32":
        wnp = np.float32
    elif W_DTYPE == "float16":
        wnp = np.float16
    else:
        import ml_dtypes

        wnp = ml_dtypes.bfloat16

    slots_per_core, K = _route(y)
    K = ((K + DMA_GROUP - 1) // DMA_GROUP) * DMA_GROUP
    bias_zero = not np.any(b1)
    key = (K, W_DTYPE, bias_zero)
    if key not in _PROGRAM_CACHE:
        _PROGRAM_CACHE[key] = _build_program(K, bias_zero)
    nc = _PROGRAM_CACHE[key]

    Zt = np.ascontiguousarray(Z.T)  # [F, B]
    G = DMA_GROUP
    C = FT * H
    NG = K // G
    in_maps = []
    for m in range(N_CORES):
        slots = slots_per_core[m]
        cls_list = np.array(
            [c for c, _ in slots] + [0] * (K - len(slots)), dtype=np.int64
        )
        # Weight stream: wz[j, p, g*C + ft*H + h] = W1[cls_{jG+g}, h, ft*128+p]
        wzm = np.empty((NG, 128, G * C), wnp)
        np.copyto(
            wzm.reshape(NG, 128, G, FT, H),
            W1[cls_list].reshape(NG, G, H, FT, 128).transpose(0, 4, 1, 3, 2),
            casting="same_kind",
        )
        # Resident routed Z: zs[p, ft, k, s] = Z[sample_s_of_slot_k, ft*128+p]
        zsm = np.zeros((128, FT, K, S), wnp)
        for k, (_, sidx) in enumerate(slots):
            zsm[:, :, k, : len(sidx)] = (
                Zt[:, sidx].reshape(FT, 128, len(sidx)).transpose(1, 0, 2)
            )
        # b1s[p, k*HT + ht] = b1[cls_k, ht*128 + p]; same layout for w2s
        w2sm = (
            W2[cls_list]
            .reshape(K, HT, 128)
            .transpose(2, 0, 1)
            .astype(wnp)
            .reshape(128, K * HT)
        )
        im = {"wz": wzm, "zs": zsm.reshape(128, FT * K * S), "w2s": w2sm}
        if not bias_zero:
            im["b1s"] = np.ascontiguousarray(
                b1[cls_list].reshape(K, HT, 128).transpose(2, 0, 1)
            ).reshape(128, K * HT)
        in_maps.append(im)

    trace = os.environ.get("KERNEL_TRACE", "0") == "1"
    if trace:
        _install_profile_hook()
    res = run_bass_kernel_spmd(
        nc, in_maps, core_ids=list(range(N_CORES)), trace=trace
    )
    LAST_EXEC_TIME_NS = res.exec_time_ns
    LAST_MEAN_EXEC_TIME_NS = res.mean_exec_time_ns

    HS = HT * S
    out = np.empty(B, dtype=np.float32)
    for m in range(N_CORES):
        # o[ht, k, ht2, s]: slot k's layer-2 matmul block; the block
        # diagonal ht2 == ht holds W2[cls,ht-block] . th[ht-block, s].
        o = np.asarray(res.results[m]["out"]).reshape(HT, K, HT, S)
        osum = np.einsum("hkhs->ks", o)
        for k, (_, sidx) in enumerate(slots_per_core[m]):
            out[sidx] = osum[k, : len(sidx)]
    out += b2[y]
    return out
